# revision 24
# baseline (speedup 1.0000x reference)
"""Trainium2 Bass kernel for nn_DGG_LearnableK_Small.

The reference collapses analytically (see baseline notes):
  - softmax over a size-1 axis == 1, so log_p == 0 and edge_prob == 1/N
    exactly; stable argsort of a constant row is the identity permutation.
    idxs is therefore the input-independent constant iota [B,N,N] and is
    assembled on the host.
  - adj_hard[b,i,j] = sigmoid(cke - 7j + sum_l s_l relu(z_l + b1f_l)),
    z = x @ W1f, where the linear tail is folded on the host:
      wv7 = W2 @ (7 Wkp),  s = sign(wv7),  aw = |wv7|,
      W1f = W1*aw, b1f = b1*aw, cke = 2 + 7*(b2@Wkp + bkp).
    sigmoid underflows to exactly 0.0f for j >= CUT=16 at any plausible
    shift; only 16 adj columns are computed, the rest are host zeros.

Device program (per core, 1024 rows), transposed L-on-partition layout:
  PE:   4 z-matmuls  z[l, r] (lhsT = W1f chunk [128d,128l], rhs = xT
        [128d,512r], PSUM [128,512] f32) + 4 k-sum matmuls
        (lhsT = S16 [128l,16] = sign replicated 16x, rhs = y bf16) that
        both reduce over l AND broadcast the per-row logit shift to the
        16 output partitions: pk[i, r] = sum_l s_l y[l, r] for all i.
  DVE:  y = max(z, -b1f) per tile ([128,512] PSUM->SBUF bf16); the
        missing +b1f rotates into the sigmoid bias as
        C = sum_l s_l b1f_l (host constant).
  ACT:  2 sigmoids [16,512]: adjT = sigmoid(pk + bias), bias[j] =
        cke + C - 7j per-partition.  A dependency-free dummy sigmoid at
        the top of the ACT queue hoists the ACT_TABLE_LOADs off the
        critical path (they run during the input DMAs).
  DMA:  row-half 0 is computed first end-to-end (both its z matmuls only
        need the first px half), so sigmoid 0's bf16 output half leaves
        on the scalar ring while row-half 1 is still in flight.
"""

import os

import numpy as np

B, N, D, L = 4, 2048, 128, 256
NCORES = 8
ROWS = B * N          # 8192
RPC = ROWS // NCORES  # 1024 rows per core
P = 128
HALF = RPC // 2       # 512 rows per row-half (one PSUM bank of f32)
INTERVAL = 7.0
HS_START = 2.0
CUT = 16              # adj columns actually computed (rest stay 0)
LC = L // P           # 2 L-chunks of 128
PWC = L + LC * CUT    # pw tensor free size: W1f [128,256] + S16 [128,2*16]

VARIANT = os.environ.get("DGG_VARIANT", "raw")
N_WARM = int(os.environ.get("DGG_NWARM", "6"))

# (chunk, rowhalf) y tiles computed on ACT as exact relu(z+b) instead of
# DVE max(z,-b); chosen to balance the serial DVE and ACT chains.
ACT_RELU = {(1, 0), (0, 1)}

_CACHE = {}

# Results of the last device run (exec time etc.) for the local test harness.
LAST_RESULTS = None


def _build_raw():
    """Hand-scheduled raw-Bass build: no TileContext, so no pool entry/exit
    barriers, and the input DMA feeds issue at window start.  Every
    cross-engine hazard is covered by one dedicated semaphore and every
    instruction carries at most one wait (no event-semaphore legalization).
    """
    import concourse.bacc as bacc
    import concourse.mybir as mybir

    f32 = mybir.dt.float32
    bf16 = mybir.dt.bfloat16
    AF = mybir.ActivationFunctionType

    nc = bacc.Bacc(None, target_bir_lowering=False, debug=False)
    px = nc.declare_dram_parameter("px", [P, RPC], bf16, isOutput=False)
    pw = nc.declare_dram_parameter("pw", [P, PWC], bf16, isOutput=False)
    paux = nc.declare_dram_parameter("paux", [P, 8], f32, isOutput=False)
    adjT = nc.declare_dram_parameter("adjT", [CUT, RPC], bf16, isOutput=True)

    px_sb = nc.alloc_sbuf_tensor("px_sb", [P, RPC], bf16)
    pw_sb = nc.alloc_sbuf_tensor("pw_sb", [P, PWC], bf16)
    aux_sb = nc.alloc_sbuf_tensor("aux_sb", [P, 8], f32)
    yt = [[nc.alloc_sbuf_tensor(f"y{c}{h}", [P, HALF], bf16) for h in (0, 1)]
          for c in range(LC)]
    out_sb = nc.alloc_sbuf_tensor("out_sb", [CUT, RPC], bf16)
    dsc = nc.alloc_sbuf_tensor("dsc", [1, 2], f32)

    zt = [[nc.alloc_psum_tensor(f"z{c}{h}", [P, HALF], f32) for h in (0, 1)]
          for c in range(LC)]
    pk = [nc.alloc_psum_tensor(f"pk{h}", [P, HALF], f32) for h in (0, 1)]

    s_pxA = nc.alloc_semaphore("s_pxA")
    s_pxB = nc.alloc_semaphore("s_pxB")
    s_pwA = nc.alloc_semaphore("s_pwA")
    s_pwB = nc.alloc_semaphore("s_pwB")
    s_aux = nc.alloc_semaphore("s_aux")
    s_z = nc.alloc_semaphore("s_z")
    s_yd = nc.alloc_semaphore("s_yd")
    s_ya = nc.alloc_semaphore("s_ya")
    s_pk = nc.alloc_semaphore("s_pk")
    s_sig = nc.alloc_semaphore("s_sig")
    s_out = nc.alloc_semaphore("s_out")

    # ACT queue.  Dependency-free dummy activations first: the table-load
    # pass puts both ACT_TABLE_LOADs before them, overlapping the DMAs.
    # (dsc is read uninitialized on purpose; the result is scratch.)
    # Inputs are interleaved across the two hardware DGE queues so the
    # row-0 gate (W0 + px[0:512]) completes as early as possible; s_pxA
    # and s_pxB each reach 32 when both of their quarters have landed.
    nc.scalar.activation(dsc[0:1, 1:2], dsc[0:1, 0:1], AF.Sigmoid)
    nc.scalar.activation(dsc[0:1, 1:2], dsc[0:1, 0:1], AF.Relu)
    nc.scalar.dma_start(
        out=px_sb[:, 0:256], in_=px[:, 0:256]).then_inc(s_pxA, 16)
    nc.scalar.dma_start(
        out=pw_sb[:, P:PWC], in_=pw[:, P:PWC]).then_inc(s_pwB, 16)
    nc.scalar.dma_start(
        out=px_sb[:, 768:1024], in_=px[:, 768:1024]).then_inc(s_pxB, 16)

    # SP queue: W0 first (it gates the first ldweights), then the other
    # px quarters; later the two output halves.
    nc.sync.dma_start(
        out=pw_sb[:, 0:P], in_=pw[:, 0:P]).then_inc(s_pwA, 16)
    nc.sync.dma_start(
        out=px_sb[:, 256:512], in_=px[:, 256:512]).then_inc(s_pxA, 16)
    nc.sync.dma_start(
        out=px_sb[:, 512:768], in_=px[:, 512:768]).then_inc(s_pxB, 16)

    # Pool (software-DGE) queue: the small aux vector.
    nc.gpsimd.dma_start(out=aux_sb[:], in_=paux[:]).then_inc(s_aux, 16)

    # PE queue.  The PE clock is HAM-throttled to 1.2 GHz until ~3.4us of
    # sustained activity; it would otherwise sit idle until the px data
    # lands and run the whole real burst cold.  Dependency-free warm-up
    # matmuls on scratch data fill the input-DMA window and flip the
    # clock gate to 2.4 GHz before the first real matmul.
    warm = nc.alloc_sbuf_tensor("warm", [P, HALF], bf16)
    zw = nc.alloc_psum_tensor("zw", [P, HALF], f32)
    for _ in range(N_WARM):
        nc.tensor.matmul(zw[:], lhsT=warm[:, 0:P], rhs=warm[:],
                         start=True, stop=True, skip_group_check=True)

    # z order z00, z10, z11, z01: z11 shares the W1 ldweights with z10
    # and the row-1 matmuls sit where the later px half lands.
    # s_z increments: z00=1 z10=2 z11=3 z01=4.
    nc.tensor.wait_ge(s_pwA, 16)
    nc.tensor.wait_ge(s_pxA, 32)
    nc.tensor.matmul(
        zt[0][0][:], lhsT=pw_sb[:, 0:P],
        rhs=px_sb[:, 0:HALF], start=True, stop=True,
        skip_group_check=True).then_inc(s_z, 1)
    nc.tensor.wait_ge(s_pwB, 16)
    nc.tensor.matmul(
        zt[1][0][:], lhsT=pw_sb[:, P:2 * P],
        rhs=px_sb[:, 0:HALF], start=True, stop=True,
        skip_group_check=True).then_inc(s_z, 1)
    nc.tensor.wait_ge(s_pxB, 32)
    for c in (1, 0):
        nc.tensor.matmul(
            zt[c][1][:], lhsT=pw_sb[:, c * P:(c + 1) * P],
            rhs=px_sb[:, HALF:RPC], start=True, stop=True,
            skip_group_check=True).then_inc(s_z, 1)
    # k-sums; pk1 runs its S1 term first to share pk0's S1 ldweights.
    nc.tensor.wait_ge(s_yd, 1)
    nc.tensor.matmul(pk[0][0:CUT, :], lhsT=pw_sb[:, L:L + CUT],
                     rhs=yt[0][0][:], start=True, stop=False,
                     skip_group_check=True)
    nc.tensor.wait_ge(s_ya, 1)
    nc.tensor.matmul(pk[0][0:CUT, :], lhsT=pw_sb[:, L + CUT:L + 2 * CUT],
                     rhs=yt[1][0][:], start=False, stop=True,
                     skip_group_check=True).then_inc(s_pk, 1)
    nc.tensor.wait_ge(s_yd, 2)
    nc.tensor.matmul(pk[1][0:CUT, :], lhsT=pw_sb[:, L + CUT:L + 2 * CUT],
                     rhs=yt[1][1][:], start=True, stop=False,
                     skip_group_check=True)
    nc.tensor.wait_ge(s_ya, 2)
    nc.tensor.matmul(pk[1][0:CUT, :], lhsT=pw_sb[:, L:L + CUT],
                     rhs=yt[0][1][:], start=False, stop=True,
                     skip_group_check=True).then_inc(s_pk, 1)

    # DVE queue: max-form tiles (0,0) and (1,1).
    nc.vector.wait_ge(s_aux, 16)
    nc.vector.wait_ge(s_z, 1)
    nc.vector.tensor_scalar_max(
        yt[0][0][:], zt[0][0][:], aux_sb[:, 2:3]).then_inc(s_yd, 1)
    nc.vector.wait_ge(s_z, 3)
    nc.vector.tensor_scalar_max(
        yt[1][1][:], zt[1][1][:], aux_sb[:, 3:4]).then_inc(s_yd, 1)

    # ACT queue (continued): exact relu tiles (1,0) and (0,1), then the
    # sigmoids.
    nc.scalar.wait_ge(s_aux, 16)
    nc.scalar.wait_ge(s_z, 2)
    nc.scalar.activation(
        yt[1][0][:], zt[1][0][:], AF.Relu,
        bias=aux_sb[:, 1:2], scale=1.0).then_inc(s_ya, 1)
    nc.scalar.wait_ge(s_z, 4)
    nc.scalar.activation(
        yt[0][1][:], zt[0][1][:], AF.Relu,
        bias=aux_sb[:, 0:1], scale=1.0).then_inc(s_ya, 1)
    for h in (0, 1):
        nc.scalar.wait_ge(s_pk, h + 1)
        nc.scalar.activation(
            out_sb[:, h * HALF:(h + 1) * HALF],
            pk[h][0:CUT, :],
            AF.Sigmoid,
            bias=aux_sb[0:CUT, 4 + h:5 + h],
            scale=1.0,
        ).then_inc(s_sig, 1)

    # SP queue (continued): output halves leave as soon as each sigmoid
    # lands.  No trailing data-drain wait: the NRT postamble's
    # sync_barrier + dma_rearm quiesce the rings before execution is
    # reported complete (verified against the flush-waiting variant).
    for h in (0, 1):
        nc.sync.wait_ge(s_sig, h + 1)
        nc.sync.dma_start(
            out=adjT[:, h * HALF:(h + 1) * HALF],
            in_=out_sb[:, h * HALF:(h + 1) * HALF],
        ).then_inc(s_out, 16)

    # Drop the framework's kernel-entry all-engine barrier: it only
    # guards the const-AP memsets (which nothing here races with — all
    # activation biases are APs, the dummies' results are scratch) and
    # NRT's own preamble already zeroes the semaphores.  Removing it
    # lets the input DMA feeds issue at window start.
    for blk in nc.m.functions[0].blocks:
        blk.instructions = [
            i for i in blk.instructions
            if "barrier_Pool_Activation_PE_DVE_SP" not in i.concise()
        ]

    nc.compile()
    return nc


def _build_nc(variant):
    import concourse.bacc as bacc
    import concourse.mybir as mybir
    from concourse.tile import TileContext

    f32 = mybir.dt.float32
    bf16 = mybir.dt.bfloat16
    AF = mybir.ActivationFunctionType

    # Bacc (not plain Bass): its compile() legalizes semaphore waits for the
    # TRN2 one-wait-per-instruction constraint via event semaphores.
    nc = bacc.Bacc(None, target_bir_lowering=False, debug=False)
    px = nc.declare_dram_parameter("px", [P, RPC], bf16, isOutput=False)
    pw = nc.declare_dram_parameter("pw", [P, PWC], bf16, isOutput=False)
    paux = nc.declare_dram_parameter("paux", [P, 8], f32, isOutput=False)
    adjT = nc.declare_dram_parameter("adjT", [CUT, RPC], bf16, isOutput=True)

    with TileContext(nc) as tc:
        with (
            tc.tile_pool(name="sb", bufs=1) as sbp,
            tc.tile_pool(name="ps", bufs=1, space="PSUM") as ppool,
        ):
            # Dependency-free dummy activation at the top of the ACT queue:
            # Bacc.insert_act_table_loads places the sigmoid table loads
            # right before it, so they overlap the input DMAs instead of
            # stalling the first real sigmoid.
            dsrc = sbp.tile([1, 1], f32, tag="dsrc")
            ddst = sbp.tile([1, 1], f32, tag="ddst")
            nc.vector.memset(dsrc, 0.0)
            nc.scalar.activation(ddst, dsrc, AF.Sigmoid)

            px_sb = sbp.tile([P, RPC], bf16, tag="px")
            pw_sb = sbp.tile([P, PWC], bf16, tag="pw")
            aux_sb = sbp.tile([P, 8], f32, tag="paux")
            # DMA queue feed costs ~45ns/descriptor (16 per dma_start)
            # regardless of size, so inputs go as three big transfers:
            # the px half that gates both row-0 z-matmuls leads on SP,
            # weights + the second px half on the scalar ring, and the
            # small aux vector rides the (slow, software-DGE) Pool ring.
            nc.sync.dma_start(out=px_sb[:, 0:HALF], in_=px[:, 0:HALF])
            nc.scalar.dma_start(out=pw_sb, in_=pw[:])
            nc.gpsimd.dma_start(out=aux_sb, in_=paux[:])
            nc.scalar.dma_start(out=px_sb[:, HALF:RPC], in_=px[:, HALF:RPC])

            # PSUM: four z banks + two pk banks.
            zt = [[ppool.tile([P, HALF], f32, name=f"z{c}{h}", tag=f"z{c}{h}")
                   for h in (0, 1)] for c in range(LC)]
            pk = [ppool.tile([P, HALF], f32, name=f"pk{h}", tag=f"pk{h}")
                  for h in (0, 1)]
            yt = [[sbp.tile([P, HALF], bf16, name=f"y{c}{h}", tag=f"y{c}{h}")
                   for h in (0, 1)] for c in range(LC)]
            out_sb = sbp.tile([CUT, RPC], bf16, tag="adjT")

            # All z-matmuls up front (row-half 0 first: it only needs the
            # first px half), then the y tiles, then the k-sums.  The y
            # chain is the serial tail, so one tile rides the otherwise
            # idle ACT engine as an exact relu(z+b) while DVE does the
            # max-form ones (ACT_RELU below marks which).
            for h in (0, 1):
                for c in range(LC):
                    nc.tensor.matmul(
                        zt[c][h],
                        lhsT=pw_sb[:, c * P:(c + 1) * P],
                        rhs=px_sb[:, h * HALF:(h + 1) * HALF],
                        start=True,
                        stop=True,
                        skip_group_check=True,
                    )
            for h, c in ((0, 0), (0, 1), (1, 0), (1, 1)):
                if (c, h) in ACT_RELU:
                    nc.scalar.activation(
                        yt[c][h], zt[c][h], AF.Relu,
                        bias=aux_sb[:, c:c + 1], scale=1.0)
                else:
                    nc.vector.tensor_scalar_max(
                        yt[c][h], zt[c][h], aux_sb[:, 2 + c:3 + c])
            for h in (0, 1):
                for c in range(LC):
                    nc.tensor.matmul(
                        pk[h][0:CUT, :],
                        lhsT=pw_sb[:, L + c * CUT:L + (c + 1) * CUT],
                        rhs=yt[c][h],
                        start=(c == 0),
                        stop=(c == LC - 1),
                        skip_group_check=True,
                    )
                nc.scalar.activation(
                    out_sb[:, h * HALF:(h + 1) * HALF],
                    pk[h][0:CUT, :],
                    AF.Sigmoid,
                    bias=aux_sb[0:CUT, 4 + h:5 + h],
                    scale=1.0,
                )
                # Output halves leave on the idle SP ring: a dma feed on
                # the ACT queue between the sigmoids would stall sigmoid 1.
                nc.sync.dma_start(
                    out=adjT[:, h * HALF:(h + 1) * HALF],
                    in_=out_sb[:, h * HALF:(h + 1) * HALF],
                )

    nc.compile()
    return nc


def kernel(**inputs):
    global LAST_RESULTS
    import ml_dtypes
    from concourse.bass_utils import run_bass_kernel_spmd

    bf16 = ml_dtypes.bfloat16

    x = np.ascontiguousarray(np.asarray(inputs["x"], dtype=np.float32))
    W1 = np.asarray(inputs["W_mu1"], dtype=np.float32)
    b1v = np.asarray(inputs["b_mu1"], dtype=np.float32)
    W2 = np.asarray(inputs["W_mu2"], dtype=np.float32)
    b2v = np.asarray(inputs["b_mu2"], dtype=np.float32)
    Wkp = np.asarray(inputs["W_kp"], dtype=np.float32)
    bkp = np.asarray(inputs["b_kp"], dtype=np.float32)

    # Host-side folding of the linear tail (replicated across cores).
    wv7 = (W2.astype(np.float64) @ (INTERVAL * Wkp[:, 0].astype(np.float64)))
    cke = HS_START + INTERVAL * float(
        b2v.astype(np.float64) @ Wkp[:, 0].astype(np.float64)
        + np.float64(bkp[0]))
    s = np.where(wv7 > 0, 1.0, -1.0)
    aw = np.abs(wv7)
    W1f = (W1.astype(np.float64) * aw[None, :]).astype(np.float32)
    b1f = (b1v.astype(np.float64) * aw).astype(np.float64)

    variant = VARIANT
    key = ("nc", variant)
    if key not in _CACHE:
        _CACHE[key] = (_build_raw() if variant == "raw"
                       else _build_nc(variant))
    nc = _CACHE[key]

    # Sigmoid-bias correction: the max-form tiles drop +b1f, so the
    # per-row-half C_h = sum over max-form chunks of s*b1f comes back
    # via the per-partition bias.
    csb = [float((s * b1f)[c * P:(c + 1) * P].sum()) for c in range(LC)]
    Ch = [sum(csb[c] for c in range(LC) if (c, h) not in ACT_RELU)
          for h in (0, 1)]

    pw_h = np.empty((P, PWC), dtype=bf16)
    pw_h[:, 0:L] = W1f.astype(bf16)
    for c in range(LC):
        pw_h[:, L + c * CUT:L + (c + 1) * CUT] = (
            s[c * P:(c + 1) * P].astype(bf16)[:, None])

    paux_h = np.zeros((P, 8), dtype=np.float32)
    paux_h[:, 0] = b1f[0:P].astype(np.float32)
    paux_h[:, 1] = b1f[P:2 * P].astype(np.float32)
    paux_h[:, 2] = (-b1f[0:P]).astype(np.float32)
    paux_h[:, 3] = (-b1f[P:2 * P]).astype(np.float32)
    js = np.arange(CUT, dtype=np.float64)
    paux_h[0:CUT, 4] = (cke + Ch[0] - INTERVAL * js).astype(np.float32)
    paux_h[0:CUT, 5] = (cke + Ch[1] - INTERVAL * js).astype(np.float32)

    x_flat = x.reshape(ROWS, D)
    in_maps = []
    for c in range(NCORES):
        pxc = np.ascontiguousarray(
            x_flat[c * RPC:(c + 1) * RPC].T).astype(bf16)
        in_maps.append({"px": pxc, "pw": pw_h, "paux": paux_h})

    try:
        res = run_bass_kernel_spmd(nc, in_maps, list(range(NCORES)))
    except ModuleNotFoundError:
        # BASS_TRACE was set in an environment without the axon NTFF hook
        # module; retry with tracing forced off.
        os.environ["BASS_NEVER_TRACE"] = "1"
        res = run_bass_kernel_spmd(nc, in_maps, list(range(NCORES)))
    LAST_RESULTS = res

    adj_full = np.zeros((ROWS, N), dtype=np.float32)
    for c in range(NCORES):
        adj_full[c * RPC:(c + 1) * RPC, 0:CUT] = (
            res.results[c]["adjT"].astype(np.float32).T)
    idx_full = np.broadcast_to(
        np.arange(N, dtype=np.int32), (B, N, N)).copy()

    return adj_full.reshape(B, N, N), idx_full


# revision 25
# speedup vs baseline: 1.0137x; 1.0137x over previous
"""Trainium2 Bass kernel for nn_DGG_LearnableK_Small.

The reference collapses analytically (see baseline notes):
  - softmax over a size-1 axis == 1, so log_p == 0 and edge_prob == 1/N
    exactly; stable argsort of a constant row is the identity permutation.
    idxs is therefore the input-independent constant iota [B,N,N] and is
    assembled on the host.
  - adj_hard[b,i,j] = sigmoid(cke - 7j + sum_l s_l relu(z_l + b1f_l)),
    z = x @ W1f, where the linear tail is folded on the host:
      wv7 = W2 @ (7 Wkp),  s = sign(wv7),  aw = |wv7|,
      W1f = W1*aw, b1f = b1*aw, cke = 2 + 7*(b2@Wkp + bkp).
    sigmoid underflows to exactly 0.0f for j >= CUT=16 at any plausible
    shift; only 16 adj columns are computed, the rest are host zeros.

Device program (per core, 1024 rows), transposed L-on-partition layout:
  PE:   4 z-matmuls  z[l, r] (lhsT = W1f chunk [128d,128l], rhs = xT
        [128d,512r], PSUM [128,512] f32) + 4 k-sum matmuls
        (lhsT = S16 [128l,16] = sign replicated 16x, rhs = y bf16) that
        both reduce over l AND broadcast the per-row logit shift to the
        16 output partitions: pk[i, r] = sum_l s_l y[l, r] for all i.
  DVE:  y = max(z, -b1f) per tile ([128,512] PSUM->SBUF bf16); the
        missing +b1f rotates into the sigmoid bias as
        C = sum_l s_l b1f_l (host constant).
  ACT:  2 sigmoids [16,512]: adjT = sigmoid(pk + bias), bias[j] =
        cke + C - 7j per-partition.  A dependency-free dummy sigmoid at
        the top of the ACT queue hoists the ACT_TABLE_LOADs off the
        critical path (they run during the input DMAs).
  DMA:  row-half 0 is computed first end-to-end (both its z matmuls only
        need the first px half), so sigmoid 0's bf16 output half leaves
        on the scalar ring while row-half 1 is still in flight.
"""

import os

import numpy as np

B, N, D, L = 4, 2048, 128, 256
NCORES = 8
ROWS = B * N          # 8192
RPC = ROWS // NCORES  # 1024 rows per core
P = 128
HALF = RPC // 2       # 512 rows per row-half (one PSUM bank of f32)
INTERVAL = 7.0
HS_START = 2.0
CUT = 16              # adj columns actually computed (rest stay 0)
LC = L // P           # 2 L-chunks of 128
PWC = L + LC * CUT    # pw tensor free size: W1f [128,256] + S16 [128,2*16]

VARIANT = os.environ.get("DGG_VARIANT", "raw")
N_WARM = int(os.environ.get("DGG_NWARM", "6"))

# (chunk, rowhalf) y tiles computed on ACT as exact relu(z+b) instead of
# DVE max(z,-b); chosen to balance the serial DVE and ACT chains.
ACT_RELU = {(1, 0), (0, 1)}

_CACHE = {}

# Results of the last device run (exec time etc.) for the local test harness.
LAST_RESULTS = None


def _build_raw():
    """Hand-scheduled raw-Bass build: no TileContext, so no pool entry/exit
    barriers, and the input DMA feeds issue at window start.  Every
    cross-engine hazard is covered by one dedicated semaphore and every
    instruction carries at most one wait (no event-semaphore legalization).
    """
    import concourse.bacc as bacc
    import concourse.mybir as mybir

    f32 = mybir.dt.float32
    bf16 = mybir.dt.bfloat16
    fp8 = mybir.dt.float8e4
    AF = mybir.ActivationFunctionType

    nc = bacc.Bacc(None, target_bir_lowering=False, debug=False)
    px = nc.declare_dram_parameter("px", [P, RPC], fp8, isOutput=False)
    pw = nc.declare_dram_parameter("pw", [P, PWC], bf16, isOutput=False)
    paux = nc.declare_dram_parameter("paux", [P, 8], f32, isOutput=False)
    adjT = nc.declare_dram_parameter("adjT", [CUT, RPC], bf16, isOutput=True)

    px_sb = nc.alloc_sbuf_tensor("px_sb", [P, RPC], fp8)
    pw_sb = nc.alloc_sbuf_tensor("pw_sb", [P, PWC], bf16)
    aux_sb = nc.alloc_sbuf_tensor("aux_sb", [P, 8], f32)
    yt = [[nc.alloc_sbuf_tensor(f"y{c}{h}", [P, HALF], bf16) for h in (0, 1)]
          for c in range(LC)]
    out_sb = nc.alloc_sbuf_tensor("out_sb", [CUT, RPC], bf16)
    dsc = nc.alloc_sbuf_tensor("dsc", [1, 2], f32)

    zt = [[nc.alloc_psum_tensor(f"z{c}{h}", [P, HALF], f32) for h in (0, 1)]
          for c in range(LC)]
    pk = [nc.alloc_psum_tensor(f"pk{h}", [P, HALF], f32) for h in (0, 1)]

    s_pxA = nc.alloc_semaphore("s_pxA")
    s_pxB = nc.alloc_semaphore("s_pxB")
    s_pwA = nc.alloc_semaphore("s_pwA")
    s_pwB = nc.alloc_semaphore("s_pwB")
    s_aux = nc.alloc_semaphore("s_aux")
    s_z = nc.alloc_semaphore("s_z")
    s_yd = nc.alloc_semaphore("s_yd")
    s_ya = nc.alloc_semaphore("s_ya")
    s_pk = nc.alloc_semaphore("s_pk")
    s_sig = nc.alloc_semaphore("s_sig")
    s_out = nc.alloc_semaphore("s_out")

    # ACT queue.  Dependency-free dummy activations first: the table-load
    # pass puts both ACT_TABLE_LOADs before them, overlapping the DMAs.
    # (dsc is read uninitialized on purpose; the result is scratch.)
    # Inputs are interleaved across the two hardware DGE queues so the
    # row-0 gate (W0 + px[0:512]) completes as early as possible; s_pxA
    # and s_pxB each reach 32 when both of their quarters have landed.
    nc.scalar.activation(dsc[0:1, 1:2], dsc[0:1, 0:1], AF.Sigmoid)
    nc.scalar.activation(dsc[0:1, 1:2], dsc[0:1, 0:1], AF.Relu)
    nc.scalar.dma_start(
        out=pw_sb[:, P:PWC], in_=pw[:, P:PWC]).then_inc(s_pwB, 16)
    nc.scalar.dma_start(
        out=px_sb[:, HALF:RPC], in_=px[:, HALF:RPC]).then_inc(s_pxB, 16)

    # SP queue: W0 first (it gates the first ldweights), then the other
    # px quarters; later the two output halves.
    nc.sync.dma_start(
        out=pw_sb[:, 0:P], in_=pw[:, 0:P]).then_inc(s_pwA, 16)
    nc.sync.dma_start(
        out=px_sb[:, 0:HALF], in_=px[:, 0:HALF]).then_inc(s_pxA, 16)

    # Pool (software-DGE) queue: the small aux vector.
    nc.gpsimd.dma_start(out=aux_sb[:], in_=paux[:]).then_inc(s_aux, 16)

    # PE queue.  The PE clock is HAM-throttled to 1.2 GHz until ~3.4us of
    # sustained activity; it would otherwise sit idle until the px data
    # lands and run the whole real burst cold.  Dependency-free warm-up
    # matmuls on scratch data fill the input-DMA window and flip the
    # clock gate to 2.4 GHz before the first real matmul.
    warm = nc.alloc_sbuf_tensor("warm", [P, HALF], bf16)
    zw = nc.alloc_psum_tensor("zw", [P, HALF], f32)
    for _ in range(N_WARM):
        nc.tensor.matmul(zw[:], lhsT=warm[:, 0:P], rhs=warm[:],
                         start=True, stop=True, skip_group_check=True)

    # z order z00, z10, z11, z01: z11 shares the W1 ldweights with z10
    # and the row-1 matmuls sit where the later px half lands.
    # s_z increments: z00=1 z10=2 z11=3 z01=4.
    nc.tensor.wait_ge(s_pwA, 16)
    nc.tensor.wait_ge(s_pxA, 16)
    nc.tensor.matmul(
        zt[0][0][:], lhsT=pw_sb[:, 0:P],
        rhs=px_sb[:, 0:HALF], start=True, stop=True,
        skip_group_check=True).then_inc(s_z, 1)
    nc.tensor.wait_ge(s_pwB, 16)
    nc.tensor.matmul(
        zt[1][0][:], lhsT=pw_sb[:, P:2 * P],
        rhs=px_sb[:, 0:HALF], start=True, stop=True,
        skip_group_check=True).then_inc(s_z, 1)
    nc.tensor.wait_ge(s_pxB, 16)
    for c in (1, 0):
        nc.tensor.matmul(
            zt[c][1][:], lhsT=pw_sb[:, c * P:(c + 1) * P],
            rhs=px_sb[:, HALF:RPC], start=True, stop=True,
            skip_group_check=True).then_inc(s_z, 1)
    # k-sums; pk1 runs its S1 term first to share pk0's S1 ldweights.
    nc.tensor.wait_ge(s_yd, 1)
    nc.tensor.matmul(pk[0][0:CUT, :], lhsT=pw_sb[:, L:L + CUT],
                     rhs=yt[0][0][:], start=True, stop=False,
                     skip_group_check=True)
    nc.tensor.wait_ge(s_ya, 1)
    nc.tensor.matmul(pk[0][0:CUT, :], lhsT=pw_sb[:, L + CUT:L + 2 * CUT],
                     rhs=yt[1][0][:], start=False, stop=True,
                     skip_group_check=True).then_inc(s_pk, 1)
    nc.tensor.wait_ge(s_yd, 2)
    nc.tensor.matmul(pk[1][0:CUT, :], lhsT=pw_sb[:, L + CUT:L + 2 * CUT],
                     rhs=yt[1][1][:], start=True, stop=False,
                     skip_group_check=True)
    nc.tensor.wait_ge(s_ya, 2)
    nc.tensor.matmul(pk[1][0:CUT, :], lhsT=pw_sb[:, L:L + CUT],
                     rhs=yt[0][1][:], start=False, stop=True,
                     skip_group_check=True).then_inc(s_pk, 1)

    # DVE queue: max-form tiles (0,0) and (1,1).
    nc.vector.wait_ge(s_aux, 16)
    nc.vector.wait_ge(s_z, 1)
    nc.vector.tensor_scalar_max(
        yt[0][0][:], zt[0][0][:], aux_sb[:, 2:3]).then_inc(s_yd, 1)
    nc.vector.wait_ge(s_z, 3)
    nc.vector.tensor_scalar_max(
        yt[1][1][:], zt[1][1][:], aux_sb[:, 3:4]).then_inc(s_yd, 1)

    # ACT queue (continued): exact relu tiles (1,0) and (0,1), then the
    # sigmoids.
    nc.scalar.wait_ge(s_aux, 16)
    nc.scalar.wait_ge(s_z, 2)
    nc.scalar.activation(
        yt[1][0][:], zt[1][0][:], AF.Relu,
        bias=aux_sb[:, 1:2], scale=1.0).then_inc(s_ya, 1)
    nc.scalar.wait_ge(s_z, 4)
    nc.scalar.activation(
        yt[0][1][:], zt[0][1][:], AF.Relu,
        bias=aux_sb[:, 0:1], scale=1.0).then_inc(s_ya, 1)
    for h in (0, 1):
        nc.scalar.wait_ge(s_pk, h + 1)
        nc.scalar.activation(
            out_sb[:, h * HALF:(h + 1) * HALF],
            pk[h][0:CUT, :],
            AF.Sigmoid,
            bias=aux_sb[0:CUT, 4 + h:5 + h],
            scale=1.0,
        ).then_inc(s_sig, 1)

    # SP queue (continued): output halves leave as soon as each sigmoid
    # lands.  No trailing data-drain wait: the NRT postamble's
    # sync_barrier + dma_rearm quiesce the rings before execution is
    # reported complete (verified against the flush-waiting variant).
    for h in (0, 1):
        nc.sync.wait_ge(s_sig, h + 1)
        nc.sync.dma_start(
            out=adjT[:, h * HALF:(h + 1) * HALF],
            in_=out_sb[:, h * HALF:(h + 1) * HALF],
        ).then_inc(s_out, 16)

    # Drop the framework's kernel-entry all-engine barrier: it only
    # guards the const-AP memsets (which nothing here races with — all
    # activation biases are APs, the dummies' results are scratch) and
    # NRT's own preamble already zeroes the semaphores.  Removing it
    # lets the input DMA feeds issue at window start.
    for blk in nc.m.functions[0].blocks:
        blk.instructions = [
            i for i in blk.instructions
            if "barrier_Pool_Activation_PE_DVE_SP" not in i.concise()
        ]

    nc.compile()
    return nc


def _build_nc(variant):
    import concourse.bacc as bacc
    import concourse.mybir as mybir
    from concourse.tile import TileContext

    f32 = mybir.dt.float32
    bf16 = mybir.dt.bfloat16
    AF = mybir.ActivationFunctionType

    # Bacc (not plain Bass): its compile() legalizes semaphore waits for the
    # TRN2 one-wait-per-instruction constraint via event semaphores.
    nc = bacc.Bacc(None, target_bir_lowering=False, debug=False)
    px = nc.declare_dram_parameter("px", [P, RPC], bf16, isOutput=False)
    pw = nc.declare_dram_parameter("pw", [P, PWC], bf16, isOutput=False)
    paux = nc.declare_dram_parameter("paux", [P, 8], f32, isOutput=False)
    adjT = nc.declare_dram_parameter("adjT", [CUT, RPC], bf16, isOutput=True)

    with TileContext(nc) as tc:
        with (
            tc.tile_pool(name="sb", bufs=1) as sbp,
            tc.tile_pool(name="ps", bufs=1, space="PSUM") as ppool,
        ):
            # Dependency-free dummy activation at the top of the ACT queue:
            # Bacc.insert_act_table_loads places the sigmoid table loads
            # right before it, so they overlap the input DMAs instead of
            # stalling the first real sigmoid.
            dsrc = sbp.tile([1, 1], f32, tag="dsrc")
            ddst = sbp.tile([1, 1], f32, tag="ddst")
            nc.vector.memset(dsrc, 0.0)
            nc.scalar.activation(ddst, dsrc, AF.Sigmoid)

            px_sb = sbp.tile([P, RPC], bf16, tag="px")
            pw_sb = sbp.tile([P, PWC], bf16, tag="pw")
            aux_sb = sbp.tile([P, 8], f32, tag="paux")
            # DMA queue feed costs ~45ns/descriptor (16 per dma_start)
            # regardless of size, so inputs go as three big transfers:
            # the px half that gates both row-0 z-matmuls leads on SP,
            # weights + the second px half on the scalar ring, and the
            # small aux vector rides the (slow, software-DGE) Pool ring.
            nc.sync.dma_start(out=px_sb[:, 0:HALF], in_=px[:, 0:HALF])
            nc.scalar.dma_start(out=pw_sb, in_=pw[:])
            nc.gpsimd.dma_start(out=aux_sb, in_=paux[:])
            nc.scalar.dma_start(out=px_sb[:, HALF:RPC], in_=px[:, HALF:RPC])

            # PSUM: four z banks + two pk banks.
            zt = [[ppool.tile([P, HALF], f32, name=f"z{c}{h}", tag=f"z{c}{h}")
                   for h in (0, 1)] for c in range(LC)]
            pk = [ppool.tile([P, HALF], f32, name=f"pk{h}", tag=f"pk{h}")
                  for h in (0, 1)]
            yt = [[sbp.tile([P, HALF], bf16, name=f"y{c}{h}", tag=f"y{c}{h}")
                   for h in (0, 1)] for c in range(LC)]
            out_sb = sbp.tile([CUT, RPC], bf16, tag="adjT")

            # All z-matmuls up front (row-half 0 first: it only needs the
            # first px half), then the y tiles, then the k-sums.  The y
            # chain is the serial tail, so one tile rides the otherwise
            # idle ACT engine as an exact relu(z+b) while DVE does the
            # max-form ones (ACT_RELU below marks which).
            for h in (0, 1):
                for c in range(LC):
                    nc.tensor.matmul(
                        zt[c][h],
                        lhsT=pw_sb[:, c * P:(c + 1) * P],
                        rhs=px_sb[:, h * HALF:(h + 1) * HALF],
                        start=True,
                        stop=True,
                        skip_group_check=True,
                    )
            for h, c in ((0, 0), (0, 1), (1, 0), (1, 1)):
                if (c, h) in ACT_RELU:
                    nc.scalar.activation(
                        yt[c][h], zt[c][h], AF.Relu,
                        bias=aux_sb[:, c:c + 1], scale=1.0)
                else:
                    nc.vector.tensor_scalar_max(
                        yt[c][h], zt[c][h], aux_sb[:, 2 + c:3 + c])
            for h in (0, 1):
                for c in range(LC):
                    nc.tensor.matmul(
                        pk[h][0:CUT, :],
                        lhsT=pw_sb[:, L + c * CUT:L + (c + 1) * CUT],
                        rhs=yt[c][h],
                        start=(c == 0),
                        stop=(c == LC - 1),
                        skip_group_check=True,
                    )
                nc.scalar.activation(
                    out_sb[:, h * HALF:(h + 1) * HALF],
                    pk[h][0:CUT, :],
                    AF.Sigmoid,
                    bias=aux_sb[0:CUT, 4 + h:5 + h],
                    scale=1.0,
                )
                # Output halves leave on the idle SP ring: a dma feed on
                # the ACT queue between the sigmoids would stall sigmoid 1.
                nc.sync.dma_start(
                    out=adjT[:, h * HALF:(h + 1) * HALF],
                    in_=out_sb[:, h * HALF:(h + 1) * HALF],
                )

    nc.compile()
    return nc


def kernel(**inputs):
    global LAST_RESULTS
    import ml_dtypes
    from concourse.bass_utils import run_bass_kernel_spmd

    bf16 = ml_dtypes.bfloat16
    fp8 = ml_dtypes.float8_e4m3

    x = np.ascontiguousarray(np.asarray(inputs["x"], dtype=np.float32))
    W1 = np.asarray(inputs["W_mu1"], dtype=np.float32)
    b1v = np.asarray(inputs["b_mu1"], dtype=np.float32)
    W2 = np.asarray(inputs["W_mu2"], dtype=np.float32)
    b2v = np.asarray(inputs["b_mu2"], dtype=np.float32)
    Wkp = np.asarray(inputs["W_kp"], dtype=np.float32)
    bkp = np.asarray(inputs["b_kp"], dtype=np.float32)

    # Host-side folding of the linear tail (replicated across cores).
    wv7 = (W2.astype(np.float64) @ (INTERVAL * Wkp[:, 0].astype(np.float64)))
    cke = HS_START + INTERVAL * float(
        b2v.astype(np.float64) @ Wkp[:, 0].astype(np.float64)
        + np.float64(bkp[0]))
    s = np.where(wv7 > 0, 1.0, -1.0)
    aw = np.abs(wv7)
    W1f = (W1.astype(np.float64) * aw[None, :]).astype(np.float32)
    b1f = (b1v.astype(np.float64) * aw).astype(np.float64)

    variant = VARIANT
    key = ("nc", variant)
    if key not in _CACHE:
        _CACHE[key] = (_build_raw() if variant == "raw"
                       else _build_nc(variant))
    nc = _CACHE[key]

    # Sigmoid-bias correction: the max-form tiles drop +b1f, so the
    # per-row-half C_h = sum over max-form chunks of s*b1f comes back
    # via the per-partition bias.
    csb = [float((s * b1f)[c * P:(c + 1) * P].sum()) for c in range(LC)]
    Ch = [sum(csb[c] for c in range(LC) if (c, h) not in ACT_RELU)
          for h in (0, 1)]

    pw_h = np.empty((P, PWC), dtype=bf16)
    pw_h[:, 0:L] = W1f.astype(bf16)
    for c in range(LC):
        pw_h[:, L + c * CUT:L + (c + 1) * CUT] = (
            s[c * P:(c + 1) * P].astype(bf16)[:, None])

    paux_h = np.zeros((P, 8), dtype=np.float32)
    paux_h[:, 0] = b1f[0:P].astype(np.float32)
    paux_h[:, 1] = b1f[P:2 * P].astype(np.float32)
    paux_h[:, 2] = (-b1f[0:P]).astype(np.float32)
    paux_h[:, 3] = (-b1f[P:2 * P]).astype(np.float32)
    js = np.arange(CUT, dtype=np.float64)
    paux_h[0:CUT, 4] = (cke + Ch[0] - INTERVAL * js).astype(np.float32)
    paux_h[0:CUT, 5] = (cke + Ch[1] - INTERVAL * js).astype(np.float32)

    x_flat = x.reshape(ROWS, D)
    in_maps = []
    for c in range(NCORES):
        pxc = np.ascontiguousarray(
            x_flat[c * RPC:(c + 1) * RPC].T).astype(fp8)
        in_maps.append({"px": pxc, "pw": pw_h, "paux": paux_h})

    try:
        res = run_bass_kernel_spmd(nc, in_maps, list(range(NCORES)))
    except ModuleNotFoundError:
        # BASS_TRACE was set in an environment without the axon NTFF hook
        # module; retry with tracing forced off.
        os.environ["BASS_NEVER_TRACE"] = "1"
        res = run_bass_kernel_spmd(nc, in_maps, list(range(NCORES)))
    LAST_RESULTS = res

    adj_full = np.zeros((ROWS, N), dtype=np.float32)
    for c in range(NCORES):
        adj_full[c * RPC:(c + 1) * RPC, 0:CUT] = (
            res.results[c]["adjT"].astype(np.float32).T)
    idx_full = np.broadcast_to(
        np.arange(N, dtype=np.int32), (B, N, N)).copy()

    return adj_full.reshape(B, N, N), idx_full


# revision 34
# speedup vs baseline: 1.0289x; 1.0151x over previous
"""Trainium2 Bass kernel for nn_DGG_LearnableK_Small.

The reference collapses analytically:
  - softmax over a size-1 axis == 1, so log_p == 0 and edge_prob == 1/N
    exactly; stable argsort of a constant row is the identity permutation.
    idxs is therefore the input-independent constant iota [B,N,N] and is
    assembled on the host.
  - adj_hard[b,i,j] = sigmoid(cke - 7j + sum_l s_l relu(z_l + b1f_l)),
    z = x @ W1f, where the linear tail is folded on the host:
      wv7 = W2 @ (7 Wkp),  s = sign(wv7),  aw = |wv7|,
      W1f = W1*aw, b1f = b1*aw, cke = 2 + 7*(b2@Wkp + bkp).
    sigmoid underflows to exactly 0.0f for j >= CUT=16 at any plausible
    shift; only 16 adj columns are computed, the rest are host zeros.

Device program (per core, 1024 rows = B*N/8), raw Bass (no TileContext:
no pool entry/exit barriers; the framework kernel-entry barrier is also
stripped), transposed L-on-partition layout, one dedicated semaphore per
cross-engine hazard so no instruction carries more than one wait:
  PE:   HAM warm-up first - N_WARM dependency-free scratch matmuls fill
        the input-DMA window so the clock gate is at 2.4 GHz (not the
        cold 1.2) when the real matmuls run.  Then 4 z-matmuls z[l, r]
        (lhsT = W1f chunk [128d,128l], rhs = xT [128d,512r] fp8, PSUM
        [128,512] f32) and 4 k-sum matmuls (lhsT = S16 [128l,16] = sign
        replicated 16x, rhs = y bf16) that reduce over l AND broadcast
        the per-row logit shift to the 16 output partitions.  Matmul
        order shares ldweights between neighbours (z10/z11 on W1,
        pk0b/pk1a-start on S1).
  DVE:  max-form y tiles (chunks (0,0),(1,1)): y = max(z, -b1f), PSUM ->
        SBUF bf16; the dropped +b1f rotates into the sigmoid bias as
        C_h = sum over max-form chunks of s*b1f.
  ACT:  exact relu(z+b1f) for tiles (1,0),(0,1) (balances the two
        elementwise chains), then 2 sigmoids [16,512]:
        adjT = sigmoid(pk + bias), bias[j] = cke + C_h - 7j.  Two
        dependency-free dummy activations at the queue top hoist the
        ACT_TABLE_LOAD into the input-DMA window.
  x in  fp8 e4m3 (halves the dominant input transfer; end-to-end rel
        err 2.7e-3 vs the 2e-2 gate, verified against a numpy bit-model
        and on hardware), W1f/S16 bf16, biases f32.
  DMA:  scalar+SP hardware queues carry {W0, px rows 0:512} and
        {W1+S16, px rows 512:1024}; the software Pool queue carries the
        small f32 aux vector.  Output halves leave on SP as each
        sigmoid lands; no trailing drain - the NRT postamble quiesces
        the rings (verified stable against a flush-waiting variant).
"""

import os

import numpy as np

B, N, D, L = 4, 2048, 128, 256
NCORES = 8
ROWS = B * N          # 8192
RPC = ROWS // NCORES  # 1024 rows per core
P = 128
HALF = RPC // 2       # 512 rows per row-half (one PSUM bank of f32)
INTERVAL = 7.0
HS_START = 2.0
CUT = 16              # adj columns actually computed (rest stay 0)
LC = L // P           # 2 L-chunks of 128
PWC = L + LC * CUT    # pw tensor free size: W1f [128,256] + S16 [128,2*16]

N_WARM = int(os.environ.get("DGG_NWARM", "7"))

# (chunk, rowhalf) y tiles computed on ACT as exact relu(z+b) instead of
# DVE max(z,-b); chosen to balance the serial DVE and ACT chains.
ACT_RELU = {(1, 0), (0, 1)}

_CACHE = {}

# Results of the last device run (exec time etc.) for the local test harness.
LAST_RESULTS = None


def _build_raw():
    """Hand-scheduled raw-Bass build: no TileContext, so no pool entry/exit
    barriers, and the input DMA feeds issue at window start.  Every
    cross-engine hazard is covered by one dedicated semaphore and every
    instruction carries at most one wait (no event-semaphore legalization).
    """
    import concourse.bacc as bacc
    import concourse.mybir as mybir

    f32 = mybir.dt.float32
    bf16 = mybir.dt.bfloat16
    fp8 = mybir.dt.float8e4
    AF = mybir.ActivationFunctionType

    nc = bacc.Bacc(None, target_bir_lowering=False, debug=False)
    px = nc.declare_dram_parameter("px", [P, RPC], fp8, isOutput=False)
    pw = nc.declare_dram_parameter("pw", [P, PWC], bf16, isOutput=False)
    paux = nc.declare_dram_parameter("paux", [P, 8], f32, isOutput=False)
    adjT = nc.declare_dram_parameter("adjT", [CUT, RPC], bf16, isOutput=True)

    px_sb = nc.alloc_sbuf_tensor("px_sb", [P, RPC], fp8)
    pw_sb = nc.alloc_sbuf_tensor("pw_sb", [P, PWC], bf16)
    aux_sb = nc.alloc_sbuf_tensor("aux_sb", [P, 8], f32)
    yt = [[nc.alloc_sbuf_tensor(f"y{c}{h}", [P, HALF], bf16) for h in (0, 1)]
          for c in range(LC)]
    out_sb = nc.alloc_sbuf_tensor("out_sb", [CUT, RPC], bf16)
    dsc = nc.alloc_sbuf_tensor("dsc", [1, 2], f32)

    zt = [[nc.alloc_psum_tensor(f"z{c}{h}", [P, HALF], f32) for h in (0, 1)]
          for c in range(LC)]
    pk = [nc.alloc_psum_tensor(f"pk{h}", [P, HALF], f32) for h in (0, 1)]

    s_pxA = nc.alloc_semaphore("s_pxA")
    s_pxB = nc.alloc_semaphore("s_pxB")
    s_pwA = nc.alloc_semaphore("s_pwA")
    s_pwB = nc.alloc_semaphore("s_pwB")
    s_aux = nc.alloc_semaphore("s_aux")
    s_z = nc.alloc_semaphore("s_z")
    s_yd = nc.alloc_semaphore("s_yd")
    s_ya = nc.alloc_semaphore("s_ya")
    s_pk = nc.alloc_semaphore("s_pk")
    s_sig = nc.alloc_semaphore("s_sig")
    s_out = nc.alloc_semaphore("s_out")

    # ACT queue.  Dependency-free dummy activations first: the table-load
    # pass puts the ACT_TABLE_LOADs before them, overlapping the DMAs.
    # (dsc is read uninitialized on purpose; the result is scratch.)
    nc.scalar.activation(dsc[0:1, 1:2], dsc[0:1, 0:1], AF.Sigmoid)
    nc.scalar.activation(dsc[0:1, 1:2], dsc[0:1, 0:1], AF.Relu)
    # The scalar queue wakes ~0.9us before the SP queue, so it carries
    # the critical row-0 inputs (W0, then the first px half); SP gets
    # the rest plus, later, the two output halves.
    nc.scalar.dma_start(
        out=pw_sb[:, 0:P], in_=pw[:, 0:P]).then_inc(s_pwA, 16)
    nc.scalar.dma_start(
        out=px_sb[:, 0:HALF], in_=px[:, 0:HALF]).then_inc(s_pxA, 16)
    nc.scalar.dma_start(
        out=pw_sb[:, P:PWC], in_=pw[:, P:PWC]).then_inc(s_pwB, 16)
    nc.sync.dma_start(
        out=px_sb[:, HALF:RPC], in_=px[:, HALF:RPC]).then_inc(s_pxB, 16)

    # Pool (software-DGE) queue: the small aux vector, in parallel with
    # the two hardware queues.
    nc.gpsimd.dma_start(out=aux_sb[:], in_=paux[:]).then_inc(s_aux, 16)

    # PE queue.  The PE clock is HAM-throttled to 1.2 GHz until ~3.4us of
    # sustained activity; it would otherwise sit idle until the px data
    # lands and run the whole real burst cold.  Dependency-free warm-up
    # matmuls on scratch data fill the input-DMA window and flip the
    # clock gate to 2.4 GHz before the first real matmul.
    warm = nc.alloc_sbuf_tensor("warm", [P, HALF], bf16)
    zw = nc.alloc_psum_tensor("zw", [P, HALF], f32)
    for _ in range(N_WARM):
        nc.tensor.matmul(zw[:], lhsT=warm[:, 0:P], rhs=warm[:],
                         start=True, stop=True, skip_group_check=True)

    # z order z00, z10, z11, z01: z11 shares the W1 ldweights with z10
    # and the row-1 matmuls sit where the later px half lands.
    # s_z increments: z00=1 z10=2 z11=3 z01=4.
    nc.tensor.wait_ge(s_pwA, 16)
    nc.tensor.wait_ge(s_pxA, 16)
    nc.tensor.matmul(
        zt[0][0][:], lhsT=pw_sb[:, 0:P],
        rhs=px_sb[:, 0:HALF], start=True, stop=True,
        skip_group_check=True).then_inc(s_z, 1)
    nc.tensor.wait_ge(s_pwB, 16)
    nc.tensor.matmul(
        zt[1][0][:], lhsT=pw_sb[:, P:2 * P],
        rhs=px_sb[:, 0:HALF], start=True, stop=True,
        skip_group_check=True).then_inc(s_z, 1)
    nc.tensor.wait_ge(s_pxB, 16)
    for c in (1, 0):
        nc.tensor.matmul(
            zt[c][1][:], lhsT=pw_sb[:, c * P:(c + 1) * P],
            rhs=px_sb[:, HALF:RPC], start=True, stop=True,
            skip_group_check=True).then_inc(s_z, 1)
    # k-sums; pk1 runs its S1 term first to share pk0's S1 ldweights.
    nc.tensor.wait_ge(s_yd, 1)
    nc.tensor.matmul(pk[0][0:CUT, :], lhsT=pw_sb[:, L:L + CUT],
                     rhs=yt[0][0][:], start=True, stop=False,
                     skip_group_check=True)
    nc.tensor.wait_ge(s_ya, 1)
    nc.tensor.matmul(pk[0][0:CUT, :], lhsT=pw_sb[:, L + CUT:L + 2 * CUT],
                     rhs=yt[1][0][:], start=False, stop=True,
                     skip_group_check=True).then_inc(s_pk, 1)
    nc.tensor.wait_ge(s_yd, 2)
    nc.tensor.matmul(pk[1][0:CUT, :], lhsT=pw_sb[:, L + CUT:L + 2 * CUT],
                     rhs=yt[1][1][:], start=True, stop=False,
                     skip_group_check=True)
    nc.tensor.wait_ge(s_ya, 2)
    nc.tensor.matmul(pk[1][0:CUT, :], lhsT=pw_sb[:, L:L + CUT],
                     rhs=yt[0][1][:], start=False, stop=True,
                     skip_group_check=True).then_inc(s_pk, 1)

    # DVE queue: max-form tiles (0,0) and (1,1).
    nc.vector.wait_ge(s_aux, 16)
    nc.vector.wait_ge(s_z, 1)
    nc.vector.tensor_scalar_max(
        yt[0][0][:], zt[0][0][:], aux_sb[:, 2:3]).then_inc(s_yd, 1)
    nc.vector.wait_ge(s_z, 3)
    nc.vector.tensor_scalar_max(
        yt[1][1][:], zt[1][1][:], aux_sb[:, 3:4]).then_inc(s_yd, 1)

    # ACT queue (continued): exact relu tiles (1,0) and (0,1), then the
    # sigmoids.
    nc.scalar.wait_ge(s_aux, 16)
    nc.scalar.wait_ge(s_z, 2)
    nc.scalar.activation(
        yt[1][0][:], zt[1][0][:], AF.Relu,
        bias=aux_sb[:, 1:2], scale=1.0).then_inc(s_ya, 1)
    nc.scalar.wait_ge(s_z, 4)
    nc.scalar.activation(
        yt[0][1][:], zt[0][1][:], AF.Relu,
        bias=aux_sb[:, 0:1], scale=1.0).then_inc(s_ya, 1)
    for h in (0, 1):
        nc.scalar.wait_ge(s_pk, h + 1)
        nc.scalar.activation(
            out_sb[:, h * HALF:(h + 1) * HALF],
            pk[h][0:CUT, :],
            AF.Sigmoid,
            bias=aux_sb[0:CUT, 4 + h:5 + h],
            scale=1.0,
        ).then_inc(s_sig, 1)

    # SP queue (continued): output halves leave as soon as each sigmoid
    # lands.  No trailing data-drain wait: the NRT postamble's
    # sync_barrier + dma_rearm quiesce the rings before execution is
    # reported complete (verified against the flush-waiting variant).
    for h in (0, 1):
        nc.sync.wait_ge(s_sig, h + 1)
        nc.sync.dma_start(
            out=adjT[:, h * HALF:(h + 1) * HALF],
            in_=out_sb[:, h * HALF:(h + 1) * HALF],
        ).then_inc(s_out, 16)

    # Drop the framework's kernel-entry all-engine barrier: it only
    # guards the const-AP memsets (which nothing here races with — all
    # activation biases are APs, the dummies' results are scratch) and
    # NRT's own preamble already zeroes the semaphores.  Removing it
    # lets the input DMA feeds issue at window start.
    for blk in nc.m.functions[0].blocks:
        blk.instructions = [
            i for i in blk.instructions
            if "barrier_Pool_Activation_PE_DVE_SP" not in i.concise()
        ]

    nc.compile()
    return nc


def kernel(**inputs):
    global LAST_RESULTS
    import ml_dtypes
    from concourse.bass_utils import run_bass_kernel_spmd

    bf16 = ml_dtypes.bfloat16
    fp8 = ml_dtypes.float8_e4m3

    x = np.ascontiguousarray(np.asarray(inputs["x"], dtype=np.float32))
    W1 = np.asarray(inputs["W_mu1"], dtype=np.float32)
    b1v = np.asarray(inputs["b_mu1"], dtype=np.float32)
    W2 = np.asarray(inputs["W_mu2"], dtype=np.float32)
    b2v = np.asarray(inputs["b_mu2"], dtype=np.float32)
    Wkp = np.asarray(inputs["W_kp"], dtype=np.float32)
    bkp = np.asarray(inputs["b_kp"], dtype=np.float32)

    # Host-side folding of the linear tail (replicated across cores).
    wv7 = (W2.astype(np.float64) @ (INTERVAL * Wkp[:, 0].astype(np.float64)))
    cke = HS_START + INTERVAL * float(
        b2v.astype(np.float64) @ Wkp[:, 0].astype(np.float64)
        + np.float64(bkp[0]))
    s = np.where(wv7 > 0, 1.0, -1.0)
    aw = np.abs(wv7)
    W1f = (W1.astype(np.float64) * aw[None, :]).astype(np.float32)
    b1f = (b1v.astype(np.float64) * aw).astype(np.float64)

    if "nc" not in _CACHE:
        _CACHE["nc"] = _build_raw()
    nc = _CACHE["nc"]

    # Sigmoid-bias correction: the max-form tiles drop +b1f, so the
    # per-row-half C_h = sum over max-form chunks of s*b1f comes back
    # via the per-partition bias.
    csb = [float((s * b1f)[c * P:(c + 1) * P].sum()) for c in range(LC)]
    Ch = [sum(csb[c] for c in range(LC) if (c, h) not in ACT_RELU)
          for h in (0, 1)]

    pw_h = np.empty((P, PWC), dtype=bf16)
    pw_h[:, 0:L] = W1f.astype(bf16)
    for c in range(LC):
        pw_h[:, L + c * CUT:L + (c + 1) * CUT] = (
            s[c * P:(c + 1) * P].astype(bf16)[:, None])

    paux_h = np.zeros((P, 8), dtype=np.float32)
    paux_h[:, 0] = b1f[0:P].astype(np.float32)
    paux_h[:, 1] = b1f[P:2 * P].astype(np.float32)
    paux_h[:, 2] = (-b1f[0:P]).astype(np.float32)
    paux_h[:, 3] = (-b1f[P:2 * P]).astype(np.float32)
    js = np.arange(CUT, dtype=np.float64)
    paux_h[0:CUT, 4] = (cke + Ch[0] - INTERVAL * js).astype(np.float32)
    paux_h[0:CUT, 5] = (cke + Ch[1] - INTERVAL * js).astype(np.float32)

    x_flat = x.reshape(ROWS, D)
    in_maps = []
    for c in range(NCORES):
        pxc = np.ascontiguousarray(
            x_flat[c * RPC:(c + 1) * RPC].T).astype(fp8)
        in_maps.append({"px": pxc, "pw": pw_h, "paux": paux_h})

    try:
        res = run_bass_kernel_spmd(nc, in_maps, list(range(NCORES)))
    except ModuleNotFoundError:
        # BASS_TRACE was set in an environment without the axon NTFF hook
        # module; retry with tracing forced off.
        os.environ["BASS_NEVER_TRACE"] = "1"
        res = run_bass_kernel_spmd(nc, in_maps, list(range(NCORES)))
    LAST_RESULTS = res

    adj_full = np.zeros((ROWS, N), dtype=np.float32)
    for c in range(NCORES):
        adj_full[c * RPC:(c + 1) * RPC, 0:CUT] = (
            res.results[c]["adjT"].astype(np.float32).T)
    idx_full = np.broadcast_to(
        np.arange(N, dtype=np.int32), (B, N, N)).copy()

    return adj_full.reshape(B, N, N), idx_full
